# revision 33
# baseline (speedup 1.0000x reference)
"""Multi-head causal attention with RoPE on 8 Trainium2 NeuronCores.

Sharding: core c -> (batch b = c//4, head-group g = c%4, heads 4g..4g+4).
wq/wk/wv column-sharded by head, wo row-sharded; attention fully local.
Host sums the 4 per-core partial output projections per batch.

Schedule: weights/trig resident in SBUF; per rep stream x in (chunked),
project k0,q0 (+RoPE on DVE) then v then k1,q1, run attention in two
head-pairs. Within a pair the two heads' QK matmuls are interleaved at
adjacent row groups (K=64 each) so the PE runs them concurrently; PV is
emitted per output 512-column group (jg) so each PSUM accumulator is
unloaded, reciprocal'd and normalized as soon as it completes. The output
projection for column group jg is interleaved into the second pair's
attention stream one group behind the normalize.

Numerics: matmul operands fp16, accumulation fp32 (PSUM), RoPE trig
tables precomputed on host (fp16). Rel err vs fp32 reference ~1e-3.
"""
import sys
sys.path.insert(0, "/opt/trn_rl_repo")
import numpy as np

import concourse.bass as bass
import concourse.tile as tile
from concourse import bacc, mybir
from concourse.bass_utils import run_bass_kernel_spmd

F = mybir.ActivationFunctionType
A = mybir.AluOpType
FP32 = mybir.dt.float32
FP16 = mybir.dt.float16

B, D, H = 2, 1024, 16
NCORES = 8
GROUPS = 4            # head groups (cores per batch)
HL = H // GROUPS      # heads per core = 4
DK = D // H           # 64
JL = HL * DK          # local projection width = 256
ROPE_THETA = 10000.0


def build_mha(S: int, max_phase: int = 9, reps: int = 1):
    """One SPMD program: per-core shard of the full MHA layer."""
    assert S % 512 == 0
    NT = S // 128          # 128-tiles along sequence
    NC = S // 512          # 512-chunks along sequence
    KT = D // 128          # 8 contraction tiles for projections
    SCALE = 1.0 / np.sqrt(DK)

    nc = bacc.Bacc(None, target_bir_lowering=False, debug=False)

    xt_in = nc.declare_dram_parameter("xt", [D, S], FP16, isOutput=False)
    wq_in = nc.declare_dram_parameter("wqt", [D, JL], FP16, isOutput=False)
    wk_in = nc.declare_dram_parameter("wkt", [D, JL], FP16, isOutput=False)
    wv_in = nc.declare_dram_parameter("wvt", [D, JL], FP16, isOutput=False)
    wo_in = nc.declare_dram_parameter("wot", [JL, D], FP16, isOutput=False)
    cos_in = nc.declare_dram_parameter("cos64", [DK, S], FP16, isOutput=False)
    sin_in = nc.declare_dram_parameter("sinalt64", [DK, S], FP16, isOutput=False)
    ind_in = nc.declare_dram_parameter("indicator", [2, 128], FP16, isOutput=False)
    y_out = nc.declare_dram_parameter("y", [S, D], FP16, isOutput=True)

    with tile.TileContext(nc) as tc:
        # resident tensors: weights, trig tables, and cross-phase operands
        persist = tc.alloc_tile_pool(name="persist", bufs=1)
        qTb = [persist.tile([128, S], FP16, tag=f"qTb{i}", name=f"qTb{i}") for i in range(2)]
        kTb = [persist.tile([128, S], FP16, tag=f"kTb{i}", name=f"kTb{i}") for i in range(2)]
        v_sb = persist.tile([128, NT, HL, DK + 1], FP16, tag="v")
        attnT = [persist.tile([128, S], FP16, tag=f"aT{i}", name=f"aT{i}") for i in range(2)]
        woTb = persist.tile([128, 2, D], FP16, tag="woTb")
        # denominator rows packed at legal partition bases 0/32/64/96:
        # row 32*(2*jt + hpar) holds 1/den for head 2*jt+hpar
        den_sb = persist.tile([128, S], FP16, tag="den_sb")
        # indicator rows at partition bases matching den_sb rows
        ind4 = persist.tile([128, 128], FP16, tag="ind4")
        cos128 = persist.tile([128, S], FP16, tag="cos128")
        sinalt128 = persist.tile([128, S], FP16, tag="sinalt128")
        # SP queue: k/q weights then x chunks (first compute). Activation
        # queue: trig tables (needed at RoPE ~10us), then wo/v weights.
        wb = {}
        for name, win in (("k", wk_in), ("q", wq_in), ("v", wv_in)):
            wb[name] = persist.tile([128, KT, JL], FP16, tag=f"wb{name}", name=f"wb{name}")
            if name != "v":
                nc.sync.dma_start(out=wb[name],
                                  in_=win[:, :].rearrange("(k p) j -> p k j", p=128))
        for jt_ in range(2):
            nc.sync.dma_start(out=ind4[64 * jt_:64 * jt_ + 1, :], in_=ind_in[0:1, :])
            nc.sync.dma_start(out=ind4[64 * jt_ + 32:64 * jt_ + 33, :], in_=ind_in[1:2, :])
        nc.scalar.dma_start(out=cos128[0:DK, :], in_=cos_in[:, :])
        nc.scalar.dma_start(out=cos128[DK:128, :], in_=cos_in[:, :])
        nc.scalar.dma_start(out=sinalt128[0:DK, :], in_=sin_in[:, :])
        nc.scalar.dma_start(out=sinalt128[DK:128, :], in_=sin_in[:, :])
        nc.scalar.dma_start(out=woTb, in_=wo_in[:, :].rearrange("(t p) e -> p t e", p=128))
        nc.scalar.dma_start(out=wb["v"], in_=wv_in[:, :].rearrange("(k p) j -> p k j", p=128))
        # softmax-denominator ones column of v (constant)
        for hh in range(HL):
            nc.vector.memset(v_sb[:, :, hh, DK:DK + 1], 1.0)

        for _rep in range(reps):
            with tc.tile_pool(name="es", bufs=1) as es_pool, \
                 tc.tile_pool(name="sp_ps", bufs=2, space="PSUM") as sp_pool, \
                 tc.tile_pool(name="ov_ps", bufs=1, space="PSUM") as ov_pool, \
                 tc.tile_pool(name="ysb", bufs=2) as ysb:

                pps = None     # proj PSUM pool, bound below
                ropep = None   # rope staging pool, bound below
                xtb = None     # x staging tile, bound below

                def project_qk(jt, name):
                    dstpair = kTb if name == "k" else qTb
                    t16 = ropep.tile([128, S], FP16, tag="t16")
                    swp = ropep.tile([128, S], FP16, tag="swp")
                    tmp = ropep.tile([128, S], FP16, tag="ropetmp", bufs=1)
                    for sc in range(NC):
                        ps = pps.tile([128, 512], FP32, tag="projps")
                        for k in range(KT):
                            nc.tensor.matmul(
                                out=ps,
                                lhsT=wb[name][:, k, 128 * jt:128 * (jt + 1)],
                                rhs=xtb[:, k, 512 * sc:512 * (sc + 1)],
                                start=(k == 0), stop=(k == KT - 1))
                        ss = slice(512 * sc, 512 * (sc + 1))
                        if jt == 0:
                            nc.scalar.activation(out=t16[:, ss], in_=ps, func=F.Copy)
                        else:
                            nc.vector.tensor_copy(out=t16[:, ss], in_=ps)
                    # RoPE (per 64-row head block: 32 even-d rows then 32 odd-d);
                    # block swap via DVE copies (4x mode), muls/add in fp16 (2x)
                    for blk in range(4):
                        src_b, dst_b = 32 * (blk ^ 1), 32 * blk
                        nc.vector.tensor_copy(out=swp[dst_b:dst_b + 32, :],
                                              in_=t16[src_b:src_b + 32, :])
                    nc.vector.tensor_mul(tmp, t16, cos128)
                    nc.vector.tensor_mul(swp, swp, sinalt128)
                    nc.vector.tensor_add(dstpair[jt], tmp, swp)

                def project_v(st0, st1):
                    for st in range(st0, st1):
                        ps = pps.tile([128, 512], FP32, tag="projps")
                        for k in range(KT):
                            nc.tensor.matmul(out=ps[:, 0:JL],
                                             lhsT=xtb[:, k, 128 * st:128 * (st + 1)],
                                             rhs=wb["v"][:, k, :],
                                             start=(k == 0), stop=(k == KT - 1))
                        nc.vector.tensor_copy(
                            out=v_sb[:, st, :, 0:DK],
                            in_=ps[:, 0:JL].rearrange("p (h d) -> p h d", h=HL))

                # ---- attention pair (heads 2*jt, 2*jt+1) ----
                def attention_pair(jt, extra, bcpool):
                    """extra(jg) emits trailing work (second pair: output proj)."""
                    pb = {0: 0, 1: 64}
                    kTh, qTh = kTb[jt], qTb[jt]
                    esr = {}   # (hpar, mi) -> tile

                    def pv_group(jg):
                        # PV accumulation for output cols [512jg, 512jg+512)
                        ovs = {}
                        for hpar in range(2):
                            h = 2 * jt + hpar
                            ov = ov_pool.tile([DK + 1, 512], FP32,
                                              tag=f"ov{hpar}", name=f"ov{jt}_{hpar}_{jg}")
                            ovs[hpar] = ov
                            for mi in range(0, 4 * jg + 4):
                                lo = max(512 * jg, 128 * mi)
                                nc.tensor.matmul(
                                    out=ov[:, lo - 512 * jg:512],
                                    lhsT=v_sb[:, mi, h, :],
                                    rhs=esr[hpar, mi][:, lo - 128 * mi:512 * (jg + 1) - 128 * mi],
                                    start=(mi == 0), stop=(mi == 4 * jg + 3))
                        cs = slice(512 * jg, 512 * (jg + 1))
                        # unload + denominators + normalize this column group
                        for hpar in range(2):
                            nc.vector.tensor_copy(
                                out=attnT[jt][pb[hpar]:pb[hpar] + DK, cs],
                                in_=ovs[hpar][0:DK, :])
                            dr = 32 * (2 * jt + hpar)
                            with nc.allow_low_precision(reason="1/den fits fp16"):
                                nc.vector.reciprocal(
                                    out=den_sb[dr:dr + 1, cs],
                                    in_=ovs[hpar][DK:DK + 1, :])
                        bc = bcpool.tile([128, 512], FP32,
                                         tag="projps" if jt == 0 else "po",
                                         name=f"bc{jt}_{jg}")
                        for hpar in range(2):
                            dr = 32 * (2 * jt + hpar)
                            nc.tensor.matmul(out=bc, lhsT=ind4[dr:dr + 1, :],
                                             rhs=den_sb[dr:dr + 1, cs],
                                             start=(hpar == 0), stop=(hpar == 1),
                                             tile_position=(dr, 0))
                        nc.vector.tensor_mul(attnT[jt][:, cs], attnT[jt][:, cs], bc)

                    for mi in range(NT):
                        W = S - 128 * mi
                        sps = {}
                        for hpar in range(2):
                            # early-mi tags double-buffered so the second
                            # pair's softmax starts before the first pair's
                            # trailing PV groups release their esr tiles
                            esr[hpar, mi] = es_pool.tile(
                                [128, W], FP16, tag=f"esr{hpar}_{mi}",
                                name=f"esr{jt}_{hpar}_{mi}",
                                bufs=2 if mi <= 1 else 1)
                        for cb in range(0, W, 1024):
                            cw = min(1024, W - cb)
                            for hpar in range(2):
                                sps[hpar] = sp_pool.tile([128, 1024], FP32, tag="sp",
                                                         name=f"sp{jt}_{hpar}_{mi}_{cb}")
                            # interleave the two heads' QK chunks: adjacent
                            # matmuls sit on disjoint PE row groups (0-63 /
                            # 64-127) and different PSUM banks -> concurrent
                            for sb0 in range(0, cw, 512):
                                sw = min(512, cw - sb0)
                                n0 = 128 * mi + cb + sb0
                                for hpar in range(2):
                                    nc.tensor.matmul(
                                        out=sps[hpar][:, sb0:sb0 + sw],
                                        lhsT=kTh[pb[hpar]:pb[hpar] + DK, 128 * mi:128 * (mi + 1)],
                                        rhs=qTh[pb[hpar]:pb[hpar] + DK, n0:n0 + sw],
                                        start=True, stop=True)
                            for hpar in range(2):
                                nc.scalar.activation(out=esr[hpar, mi][:, cb:cb + cw],
                                                     in_=sps[hpar][:, 0:cw],
                                                     func=F.Exp, scale=SCALE)
                        # causal mask on diagonal 128 cols: keep where n-m >= 0
                        for hpar in range(2):
                            nc.gpsimd.affine_select(
                                out=esr[hpar, mi][:, 0:128], in_=esr[hpar, mi][:, 0:128],
                                pattern=[[1, 128]], compare_op=A.is_ge, fill=0.0,
                                base=0, channel_multiplier=-1)
                        if mi == 4:
                            pv_group(0)
                        elif mi == 8:
                            pv_group(1)
                            extra(0)
                        elif mi == 12:
                            pv_group(2)
                            extra(1)
                        # interleave remaining projections into pair-0 stream
                        if jt == 0:
                            if mi == 0:
                                project_v(4, 8)
                            elif mi == 1:
                                project_qk(1, "k")
                            elif mi == 2:
                                project_qk(1, "q")
                            elif mi == 3:
                                project_v(8, 12)
                            elif mi == 5:
                                project_v(12, 16)
                    pv_group(3)
                    extra(2)
                    extra(3)

                def no_extra(jg):
                    pass

                ECH = D // 512

                def out_proj(jg):
                    # output projection for seq tiles st in [4jg, 4jg+4)
                    for st in range(4 * jg, 4 * jg + 4):
                        yst = ysb.tile([128, D], FP16, tag="yst")
                        for ec in range(ECH):
                            po = po_pool.tile([128, 512], FP32, tag="po",
                                              name=f"po{st}_{ec}")
                            for jt in range(2):
                                nc.tensor.matmul(
                                    out=po,
                                    lhsT=attnT[jt][:, 128 * st:128 * (st + 1)],
                                    rhs=woTb[:, jt, 512 * ec:512 * (ec + 1)],
                                    start=(jt == 0), stop=(jt == 1))
                            if ec % 2 == 0:
                                nc.scalar.activation(out=yst[:, 512 * ec:512 * (ec + 1)],
                                                     in_=po, func=F.Copy)
                            else:
                                nc.vector.tensor_copy(out=yst[:, 512 * ec:512 * (ec + 1)],
                                                      in_=po)
                        nc.sync.dma_start(
                            out=y_out[128 * st:128 * (st + 1), :], in_=yst)

                with tc.tile_pool(name="proj", bufs=1) as proj, \
                     tc.tile_pool(name="ropep", bufs=2) as _ropep, \
                     tc.tile_pool(name="proj_ps", bufs=2, space="PSUM") as _pps:
                    pps, ropep = _pps, _ropep
                    xtb = proj.tile([128, KT, S], FP16, tag="xtb")
                    for sc in range(NC):
                        nc.sync.dma_start(
                            out=xtb[:, :, 512 * sc:512 * (sc + 1)],
                            in_=xt_in[:, 512 * sc:512 * (sc + 1)].rearrange(
                                "(k p) s -> p k s", p=128))
                    project_qk(0, "k")
                    project_qk(0, "q")
                    project_v(0, 4)
                    attention_pair(0, no_extra, _pps)
                with tc.tile_pool(name="po_ps", bufs=2, space="PSUM") as po_pool:
                    attention_pair(1, out_proj, po_pool)

        persist.release()

    nc.compile()
    return nc


_cache = {}

def _get_program(S):
    if S not in _cache:
        _cache[S] = build_mha(S)
    return _cache[S]


def make_in_maps(x, token_positions, wq, wk, wv, wo):
    S = x.shape[1]
    f16 = np.float16
    invfreq = ROPE_THETA ** (-np.arange(0, DK, 2, dtype=np.float64) / DK)  # [32]
    # perm: within each 64-wide head block, evens first then odds
    blockperm = np.concatenate([np.arange(0, DK, 2), np.arange(1, DK, 2)])
    jperm = np.concatenate([64 * hh + blockperm for hh in range(HL)])
    indicator = np.zeros((2, 128), dtype=f16)
    indicator[0, 0:64] = 1.0
    indicator[1, 64:128] = 1.0

    pos = np.asarray(token_positions, dtype=np.float64)  # [B, S]
    tables = []
    for b in range(B):
        ang = pos[b][None, :] * invfreq[:, None]          # [32, S]
        cos = np.cos(ang)
        sin = np.sin(ang)
        cos64 = np.concatenate([cos, cos], axis=0).astype(f16)       # [64, S]
        sinalt = np.concatenate([-sin, sin], axis=0).astype(f16)     # [64, S]
        tables.append((np.ascontiguousarray(cos64), np.ascontiguousarray(sinalt)))

    in_maps = []
    for c in range(NCORES):
        b, g = c // GROUPS, c % GROUPS
        js = slice(JL * g, JL * (g + 1))
        cos64, sinalt = tables[b]
        in_maps.append({
            "xt": np.ascontiguousarray(x[b].T).astype(f16),
            "wqt": np.ascontiguousarray(wq[js, :][jperm, :].T).astype(f16),
            "wkt": np.ascontiguousarray(wk[js, :][jperm, :].T).astype(f16),
            "wvt": np.ascontiguousarray(wv[js, :].T).astype(f16),
            "wot": np.ascontiguousarray(wo[:, js].T).astype(f16),
            "cos64": cos64,
            "sinalt64": sinalt,
            "indicator": indicator,
        })
    return in_maps


def kernel(x, token_positions, wq, wk, wv, wo):
    x = np.asarray(x, dtype=np.float32)
    token_positions = np.asarray(token_positions)
    wq = np.asarray(wq, dtype=np.float32)
    wk = np.asarray(wk, dtype=np.float32)
    wv = np.asarray(wv, dtype=np.float32)
    wo = np.asarray(wo, dtype=np.float32)
    S = x.shape[1]

    nc = _get_program(S)
    in_maps = make_in_maps(x, token_positions, wq, wk, wv, wo)
    res = run_bass_kernel_spmd(nc, in_maps, core_ids=list(range(NCORES)))
    out = np.zeros((B, S, D), dtype=np.float32)
    for c in range(NCORES):
        out[c // GROUPS] += res.results[c]["y"].astype(np.float32)
    return out


# revision 34
# speedup vs baseline: 1.1142x; 1.1142x over previous
"""Multi-head causal attention with RoPE on 8 Trainium2 NeuronCores.

Sharding: core c -> (batch b = c//4, head-group g = c%4, heads 4g..4g+4).
wq/wk/wv column-sharded by head, wo row-sharded; attention fully local.
Host sums the 4 per-core partial output projections per batch.

Schedule: weights/trig resident in SBUF; per rep stream x in (chunked),
project k0,q0 (+RoPE on DVE) then v then k1,q1, run attention in two
head-pairs. Within a pair the two heads' QK matmuls are interleaved at
adjacent row groups (K=64 each) so the PE runs them concurrently; PV is
emitted per output 512-column group (jg) so each PSUM accumulator is
unloaded, reciprocal'd and normalized as soon as it completes. The output
projection for column group jg is interleaved into the second pair's
attention stream one group behind the normalize.

Numerics: matmul operands fp16, accumulation fp32 (PSUM), RoPE trig
tables precomputed on host (fp16). Rel err vs fp32 reference ~1e-3.
"""
import sys
sys.path.insert(0, "/opt/trn_rl_repo")
import numpy as np

import concourse.bass as bass
import concourse.tile as tile
from concourse import bacc, mybir
from concourse.bass_utils import run_bass_kernel_spmd

F = mybir.ActivationFunctionType
A = mybir.AluOpType
FP32 = mybir.dt.float32
FP16 = mybir.dt.float16

B, D, H = 2, 1024, 16
NCORES = 8
GROUPS = 4            # head groups (cores per batch)
HL = H // GROUPS      # heads per core = 4
DK = D // H           # 64
JL = HL * DK          # local projection width = 256
ROPE_THETA = 10000.0


def build_mha(S: int, max_phase: int = 9, reps: int = 1):
    """One SPMD program: per-core shard of the full MHA layer."""
    assert S % 512 == 0
    NT = S // 128          # 128-tiles along sequence
    NC = S // 512          # 512-chunks along sequence
    KT = D // 128          # 8 contraction tiles for projections
    SCALE = 1.0 / np.sqrt(DK)

    nc = bacc.Bacc(None, target_bir_lowering=False, debug=False)

    xt_in = nc.declare_dram_parameter("xt", [D, S], FP16, isOutput=False)
    wq_in = nc.declare_dram_parameter("wqt", [D, JL], FP16, isOutput=False)
    wk_in = nc.declare_dram_parameter("wkt", [D, JL], FP16, isOutput=False)
    wv_in = nc.declare_dram_parameter("wvt", [D, JL], FP16, isOutput=False)
    wo_in = nc.declare_dram_parameter("wot", [JL, D], FP16, isOutput=False)
    cos_in = nc.declare_dram_parameter("cos64", [DK, S], FP16, isOutput=False)
    sin_in = nc.declare_dram_parameter("sinalt64", [DK, S], FP16, isOutput=False)
    ind_in = nc.declare_dram_parameter("indicator", [2, 128], FP16, isOutput=False)
    y_out = nc.declare_dram_parameter("y", [S, D], FP16, isOutput=True)

    with tile.TileContext(nc) as tc:
        # resident tensors: weights, trig tables, and cross-phase operands
        persist = tc.alloc_tile_pool(name="persist", bufs=1)
        qTb = [persist.tile([128, S], FP16, tag=f"qTb{i}", name=f"qTb{i}") for i in range(2)]
        kTb = [persist.tile([128, S], FP16, tag=f"kTb{i}", name=f"kTb{i}") for i in range(2)]
        v_sb = persist.tile([128, NT, HL, DK + 1], FP16, tag="v")
        attnT = [persist.tile([128, S], FP16, tag=f"aT{i}", name=f"aT{i}") for i in range(2)]
        woTb = persist.tile([128, 2, D], FP16, tag="woTb")
        den16 = [[persist.tile([1, S], FP16, tag=f"den16_{i}_{p}", name=f"den16_{i}_{p}")
                  for p in range(2)] for i in range(2)]
        ind_e = persist.tile([1, 128], FP16, tag="ind_e")
        ind_o = persist.tile([1, 128], FP16, tag="ind_o")
        cos128 = persist.tile([128, S], FP16, tag="cos128")
        sinalt128 = persist.tile([128, S], FP16, tag="sinalt128")
        wb = {}
        for name, win in (("v", wv_in), ("k", wk_in), ("q", wq_in)):
            wb[name] = persist.tile([128, KT, JL], FP16, tag=f"wb{name}", name=f"wb{name}")
            nc.sync.dma_start(out=wb[name],
                              in_=win[:, :].rearrange("(k p) j -> p k j", p=128))
        nc.sync.dma_start(out=ind_e, in_=ind_in[0:1, :])
        nc.sync.dma_start(out=ind_o, in_=ind_in[1:2, :])
        nc.sync.dma_start(out=cos128[0:DK, :], in_=cos_in[:, :])
        nc.sync.dma_start(out=cos128[DK:128, :], in_=cos_in[:, :])
        nc.sync.dma_start(out=sinalt128[0:DK, :], in_=sin_in[:, :])
        nc.sync.dma_start(out=sinalt128[DK:128, :], in_=sin_in[:, :])
        nc.sync.dma_start(out=woTb, in_=wo_in[:, :].rearrange("(t p) e -> p t e", p=128))
        # softmax-denominator ones column of v (constant)
        for hh in range(HL):
            nc.vector.memset(v_sb[:, :, hh, DK:DK + 1], 1.0)

        for _rep in range(reps):
            with tc.tile_pool(name="proj", bufs=1) as proj, \
                 tc.tile_pool(name="ropep", bufs=2) as ropep, \
                 tc.tile_pool(name="es", bufs=1) as es_pool, \
                 tc.tile_pool(name="sp_ps", bufs=2, space="PSUM") as sp_pool, \
                 tc.tile_pool(name="ov_ps", bufs=1, space="PSUM") as ov_pool, \
                 tc.tile_pool(name="ysb", bufs=3) as ysb:
                xtb = proj.tile([128, KT, S], FP16, tag="xtb")
                for sc in range(NC):
                    nc.sync.dma_start(
                        out=xtb[:, :, 512 * sc:512 * (sc + 1)],
                        in_=xt_in[:, 512 * sc:512 * (sc + 1)].rearrange(
                            "(k p) s -> p k s", p=128))

                pps = None   # proj PSUM pool, bound below

                def project_qk(jt, name):
                    dstpair = kTb if name == "k" else qTb
                    t16 = ropep.tile([128, S], FP16, tag="t16")
                    swp = ropep.tile([128, S], FP16, tag="swp")
                    tmp = ropep.tile([128, S], FP16, tag="ropetmp")
                    for sc in range(NC):
                        ps = pps.tile([128, 512], FP32, tag="projps")
                        for k in range(KT):
                            nc.tensor.matmul(
                                out=ps,
                                lhsT=wb[name][:, k, 128 * jt:128 * (jt + 1)],
                                rhs=xtb[:, k, 512 * sc:512 * (sc + 1)],
                                start=(k == 0), stop=(k == KT - 1))
                        ss = slice(512 * sc, 512 * (sc + 1))
                        if jt == 0:
                            nc.scalar.activation(out=t16[:, ss], in_=ps, func=F.Copy)
                        else:
                            nc.vector.tensor_copy(out=t16[:, ss], in_=ps)
                    # RoPE: perm layout (per 64-row head block: 32 even-d then odd-d)
                    for blk in range(4):
                        src_b, dst_b = 32 * (blk ^ 1), 32 * blk
                        nc.sync.dma_start(out=swp[dst_b:dst_b + 32, :],
                                          in_=t16[src_b:src_b + 32, :])
                    nc.vector.tensor_mul(tmp, t16, cos128)
                    nc.vector.tensor_mul(swp, swp, sinalt128)
                    nc.vector.tensor_add(dstpair[jt], tmp, swp)

                def project_v(st0, st1):
                    for st in range(st0, st1):
                        ps = pps.tile([128, 512], FP32, tag="projps")
                        for k in range(KT):
                            nc.tensor.matmul(out=ps[:, 0:JL],
                                             lhsT=xtb[:, k, 128 * st:128 * (st + 1)],
                                             rhs=wb["v"][:, k, :],
                                             start=(k == 0), stop=(k == KT - 1))
                        nc.vector.tensor_copy(
                            out=v_sb[:, st, :, 0:DK],
                            in_=ps[:, 0:JL].rearrange("p (h d) -> p h d", h=HL))

                # ---- attention pair (heads 2*jt, 2*jt+1) ----
                def attention_pair(jt, extra, bcpool):
                    """extra(jg) emits trailing work (second pair: output proj)."""
                    pb = {0: 0, 1: 64}
                    kTh, qTh = kTb[jt], qTb[jt]
                    esr = {}   # (hpar, mi) -> tile

                    def pv_group(jg):
                        # PV accumulation for output cols [512jg, 512jg+512)
                        ovs = {}
                        for hpar in range(2):
                            h = 2 * jt + hpar
                            ov = ov_pool.tile([DK + 1, 512], FP32,
                                              tag=f"ov{hpar}", name=f"ov{jt}_{hpar}_{jg}")
                            ovs[hpar] = ov
                            for mi in range(0, 4 * jg + 4):
                                lo = max(512 * jg, 128 * mi)
                                nc.tensor.matmul(
                                    out=ov[:, lo - 512 * jg:512],
                                    lhsT=v_sb[:, mi, h, :],
                                    rhs=esr[hpar, mi][:, lo - 128 * mi:512 * (jg + 1) - 128 * mi],
                                    start=(mi == 0), stop=(mi == 4 * jg + 3))
                        cs = slice(512 * jg, 512 * (jg + 1))
                        # unload + denominators + normalize this column group
                        for hpar in range(2):
                            nc.vector.tensor_copy(
                                out=attnT[jt][pb[hpar]:pb[hpar] + DK, cs],
                                in_=ovs[hpar][0:DK, :])
                            with nc.allow_low_precision(reason="1/den fits fp16"):
                                nc.vector.reciprocal(
                                    out=den16[jt][hpar][:, cs],
                                    in_=ovs[hpar][DK:DK + 1, :])
                        bc = bcpool.tile([128, 512], FP32,
                                         tag="projps" if jt == 0 else "po",
                                         name=f"bc{jt}_{jg}")
                        nc.tensor.matmul(out=bc, lhsT=ind_e, rhs=den16[jt][0][:, cs],
                                         start=True, stop=False)
                        nc.tensor.matmul(out=bc, lhsT=ind_o, rhs=den16[jt][1][:, cs],
                                         start=False, stop=True)
                        nc.vector.tensor_mul(attnT[jt][:, cs], attnT[jt][:, cs], bc)

                    for mi in range(NT):
                        W = S - 128 * mi
                        sps = {}
                        for hpar in range(2):
                            esr[hpar, mi] = es_pool.tile(
                                [128, W], FP16, tag=f"esr{hpar}_{mi}",
                                name=f"esr{jt}_{hpar}_{mi}")
                        for cb in range(0, W, 1024):
                            cw = min(1024, W - cb)
                            for hpar in range(2):
                                sps[hpar] = sp_pool.tile([128, 1024], FP32, tag="sp",
                                                         name=f"sp{jt}_{hpar}_{mi}_{cb}")
                            # interleave the two heads' QK chunks: adjacent
                            # matmuls sit on disjoint PE row groups (0-63 /
                            # 64-127) and different PSUM banks -> concurrent
                            for sb0 in range(0, cw, 512):
                                sw = min(512, cw - sb0)
                                n0 = 128 * mi + cb + sb0
                                for hpar in range(2):
                                    nc.tensor.matmul(
                                        out=sps[hpar][:, sb0:sb0 + sw],
                                        lhsT=kTh[pb[hpar]:pb[hpar] + DK, 128 * mi:128 * (mi + 1)],
                                        rhs=qTh[pb[hpar]:pb[hpar] + DK, n0:n0 + sw],
                                        start=True, stop=True)
                            for hpar in range(2):
                                nc.scalar.activation(out=esr[hpar, mi][:, cb:cb + cw],
                                                     in_=sps[hpar][:, 0:cw],
                                                     func=F.Exp, scale=SCALE)
                        # causal mask on diagonal 128 cols: keep where n-m >= 0
                        for hpar in range(2):
                            nc.gpsimd.affine_select(
                                out=esr[hpar, mi][:, 0:128], in_=esr[hpar, mi][:, 0:128],
                                pattern=[[1, 128]], compare_op=A.is_ge, fill=0.0,
                                base=0, channel_multiplier=-1)
                        if mi == 4:
                            pv_group(0)
                        elif mi == 8:
                            pv_group(1)
                            extra(0)
                        elif mi == 12:
                            pv_group(2)
                            extra(1)
                        # interleave remaining projections into pair-0 stream
                        if jt == 0:
                            if mi == 0:
                                project_v(4, 8)
                            elif mi == 1:
                                project_qk(1, "k")
                            elif mi == 2:
                                project_qk(1, "q")
                            elif mi == 3:
                                project_v(8, 16)
                    pv_group(3)
                    extra(2)
                    extra(3)

                def no_extra(jg):
                    pass

                ECH = D // 512

                def out_proj(jg):
                    # output projection for seq tiles st in [4jg, 4jg+4)
                    for st in range(4 * jg, 4 * jg + 4):
                        yst = ysb.tile([128, D], FP16, tag="yst")
                        for ec in range(ECH):
                            po = po_pool.tile([128, 512], FP32, tag="po",
                                              name=f"po{st}_{ec}")
                            for jt in range(2):
                                nc.tensor.matmul(
                                    out=po,
                                    lhsT=attnT[jt][:, 128 * st:128 * (st + 1)],
                                    rhs=woTb[:, jt, 512 * ec:512 * (ec + 1)],
                                    start=(jt == 0), stop=(jt == 1))
                            if ec % 2 == 0:
                                nc.scalar.activation(out=yst[:, 512 * ec:512 * (ec + 1)],
                                                     in_=po, func=F.Copy)
                            else:
                                nc.vector.tensor_copy(out=yst[:, 512 * ec:512 * (ec + 1)],
                                                      in_=po)
                        nc.sync.dma_start(
                            out=y_out[128 * st:128 * (st + 1), :], in_=yst)

                with tc.tile_pool(name="proj_ps", bufs=2, space="PSUM") as _pps:
                    pps = _pps
                    project_qk(0, "k")
                    project_qk(0, "q")
                    project_v(0, 4)
                    attention_pair(0, no_extra, _pps)
                with tc.tile_pool(name="po_ps", bufs=2, space="PSUM") as po_pool:
                    attention_pair(1, out_proj, po_pool)

        persist.release()

    nc.compile()
    return nc


_cache = {}

def _get_program(S):
    if S not in _cache:
        _cache[S] = build_mha(S)
    return _cache[S]


def make_in_maps(x, token_positions, wq, wk, wv, wo):
    S = x.shape[1]
    f16 = np.float16
    invfreq = ROPE_THETA ** (-np.arange(0, DK, 2, dtype=np.float64) / DK)  # [32]
    # perm: within each 64-wide head block, evens first then odds
    blockperm = np.concatenate([np.arange(0, DK, 2), np.arange(1, DK, 2)])
    jperm = np.concatenate([64 * hh + blockperm for hh in range(HL)])
    indicator = np.zeros((2, 128), dtype=f16)
    indicator[0, 0:64] = 1.0
    indicator[1, 64:128] = 1.0

    pos = np.asarray(token_positions, dtype=np.float64)  # [B, S]
    tables = []
    for b in range(B):
        ang = pos[b][None, :] * invfreq[:, None]          # [32, S]
        cos = np.cos(ang)
        sin = np.sin(ang)
        cos64 = np.concatenate([cos, cos], axis=0).astype(f16)       # [64, S]
        sinalt = np.concatenate([-sin, sin], axis=0).astype(f16)     # [64, S]
        tables.append((np.ascontiguousarray(cos64), np.ascontiguousarray(sinalt)))

    in_maps = []
    for c in range(NCORES):
        b, g = c // GROUPS, c % GROUPS
        js = slice(JL * g, JL * (g + 1))
        cos64, sinalt = tables[b]
        in_maps.append({
            "xt": np.ascontiguousarray(x[b].T).astype(f16),
            "wqt": np.ascontiguousarray(wq[js, :][jperm, :].T).astype(f16),
            "wkt": np.ascontiguousarray(wk[js, :][jperm, :].T).astype(f16),
            "wvt": np.ascontiguousarray(wv[js, :].T).astype(f16),
            "wot": np.ascontiguousarray(wo[:, js].T).astype(f16),
            "cos64": cos64,
            "sinalt64": sinalt,
            "indicator": indicator,
        })
    return in_maps


def kernel(x, token_positions, wq, wk, wv, wo):
    x = np.asarray(x, dtype=np.float32)
    token_positions = np.asarray(token_positions)
    wq = np.asarray(wq, dtype=np.float32)
    wk = np.asarray(wk, dtype=np.float32)
    wv = np.asarray(wv, dtype=np.float32)
    wo = np.asarray(wo, dtype=np.float32)
    S = x.shape[1]

    nc = _get_program(S)
    in_maps = make_in_maps(x, token_positions, wq, wk, wv, wo)
    res = run_bass_kernel_spmd(nc, in_maps, core_ids=list(range(NCORES)))
    out = np.zeros((B, S, D), dtype=np.float32)
    for c in range(NCORES):
        out[c // GROUPS] += res.results[c]["y"].astype(np.float32)
    return out
